# revision 96
# baseline (speedup 1.0000x reference)
"""Causal multi-head self-attention with RoPE on 8 TRN2 NeuronCores.

Sharding: batch(4) x head-group(2) -> 8 cores. Core c handles batch c//2 and
heads [8*(c%2), 8*(c%2)+8). Each core computes its partial output projection
(sum over its 8 heads' contribution); the host adds the two head-group
partials per batch. No device collectives needed.

Fully streamed single-pass schedule: after a small init (Q/K/V of chunk 0),
chunk qc's attention runs while chunk qc+1's Q/K/V projections are emitted
between its head-pair blocks -- causality guarantees attention(qc) only
needs chunks <= qc. The output projection of chunk qc-1 is likewise deferred
into chunk qc's loop so ready matmuls keep PE fed while exp/normalize drain.

Dtypes: matmul operands are bf16 (x resident in SBUF, loaded once; weights,
Q/K/V, P=exp(scores), output proj) -- on the PE this costs the same
cycles/row as f32r but halves SBUF/DMA and avoids the f32r narrow-tile
penalty. All accumulation is f32 in PSUM; softmax normalize runs in f32;
the final y is f32. Measured rel err vs the f32 reference ~= 4e-3.

On-chip layout: sequence on the free dimension everywhere.
  - Q^T/K^T [j, s] from projection matmuls (lhsT = W^T slices, rhs = x^T),
    RoPE via partition pair-swap (stream_shuffle) + cos/sin tables; the
    elementwise work is split DVE (shuffle/cos-mul, the PSUM readers) /
    GpSimd (sin-mul/add, SBUF only).
  - scores S^T [k, q] per k-tile into a merged [128, 2, SC] PSUM tile (both
    heads of the pair), one exp per k-tile on ScalarE (|scores|/8 <= ~15 so
    no max subtraction), causal boundary strips zeroed post-exp with a 0/1
    mask on DVE.
  - P@V contracts k on partitions; a ones-row appended to V yields the
    softmax denominator from the same matmul (M=65).
  - output projection contracts the 512 head-dims -> partial y^T [1024, s].
"""

import os
import sys
import time

for _p in ("/opt/trn_rl_repo", "/root/.axon_site/_ro/trn_rl_repo"):
    if _p not in sys.path and os.path.isdir(_p):
        sys.path.insert(0, _p)

import numpy as np
import concourse.bass as bass
import concourse.bacc as bacc
import concourse.mybir as mybir
import concourse.tile as tile
from concourse.bass_utils import run_bass_kernel_spmd

F32 = mybir.dt.float32
F32R = mybir.dt.float32r
BF16 = mybir.dt.bfloat16

B, S, D = 4, 2048, 1024
H, DK = 16, 64
HPC = 8            # heads per core
JC = HPC * DK      # 512 head-dims per core
N_CORES = 8
SC = 512           # q-chunk width (moving free dim)
NSC = S // SC      # 4
KT = 128           # k-tile (scores partition dim)
NKT = S // KT      # 16
DT = D // 128      # 8 contraction tiles for projections

# matmul operand dtype: "bf16" (fast, rel err ~4e-3), "f32r", or "f32"
MM_DTYPE = os.environ.get("KERNEL_MM_DTYPE", "bf16")
EBUFS = int(os.environ.get("KV_EBUFS", "17"))
SCBUFS = int(os.environ.get("KV_SCBUFS", "2"))
YBUFS = int(os.environ.get("KV_YBUFS", "2"))
XBUFS = int(os.environ.get("KV_XBUFS", "2"))
RBUFS = int(os.environ.get("KV_RBUFS", "5"))
OBUFS = int(os.environ.get("KV_OBUFS", "2"))
ROPE_SPLIT = os.environ.get("KV_ROPE_SPLIT", "1") == "1"
VCOPY_ACT = os.environ.get("KV_VCOPY_ACT", "0") == "1"
MASK_GPS = os.environ.get("KV_MASK_GPS", "0") == "1"
BLKORD = int(os.environ.get("KV_BLKORD", "1"))    # 1: ready-work pre-normalize
ROPEV = int(os.environ.get("KV_ROPEV", "2"))      # 1: pf staging, 2: direct
O2ACT = int(os.environ.get("KV_O2ACT", "3"))      # o2 copies on Act for qc<N
INIT_STREAM = os.environ.get("KV_INIT_STREAM", "0") == "1"
DMAORD = int(os.environ.get("KV_DMAORD", "2"))    # 2: chunk-0 trig early
IROPE = int(os.environ.get("KV_IROPE", "0"))      # rope add alternates to DVE:
                                                  # 0 never, 1 init, 2 always
OPDIST = int(os.environ.get("KV_OPDIST", "0"))    # deferred out-proj mts per
                                                  # hp block: 0 = [2,2,2,2],
                                                  # 1 = [4,4,0,0], 2 = [3,3,2,0]
VACT = int(os.environ.get("KV_VACT", "0"))        # V copies on Act for qc<N
                                                  # (DVE is rope-bound early)


_PAIR_SWAP = []
for _i in range(16):
    _PAIR_SWAP += [2 * _i + 1, 2 * _i]


def _emit(nc, tc, mmdt, dram, tag=""):
    """Emit the whole per-core program. `dram` maps name -> DRAM AP."""
    xT = dram["xT"]
    wq, wk, wv, wo = dram["wq"], dram["wk"], dram["wv"], dram["wo"]
    cosE, sinE, maskneg = dram["cosE"], dram["sinE"], dram["maskneg"]
    yT = dram["yT"]

    pvdt = mmdt
    EXP = mybir.ActivationFunctionType.Exp

    import contextlib
    with contextlib.ExitStack() as ctx:
        # ---- persistent tiles -------------------------------------------
        per = ctx.enter_context(tc.tile_pool(name=f"per{tag}", bufs=1))
        KTt = [per.tile([128, S], mmdt, tag=f"KT{j}{tag}", name=f"KT{j}{tag}") for j in range(4)]
        vo = [per.tile([128, HPC, 65], pvdt, tag=f"vo{i}{tag}", name=f"vo{i}{tag}") for i in range(NKT)]
        ones_sb = per.tile([128, HPC], F32, tag=f"ones{tag}", name=f"ones{tag}")
        cos_sb = per.tile([128, S], F32, tag=f"cos{tag}", name=f"cos{tag}")
        sin_sb = per.tile([128, S], F32, tag=f"sin{tag}", name=f"sin{tag}")
        mask_f = per.tile([128, 2, 128], F32, tag=f"maskf{tag}", name=f"maskf{tag}")
        mask_sb = per.tile([128, 2, 128], pvdt, tag=f"mask{tag}", name=f"mask{tag}")
        wq_sb = per.tile([128, DT, JC], mmdt, tag=f"wq{tag}", name=f"wq{tag}")
        wk_sb = per.tile([128, DT, JC], mmdt, tag=f"wk{tag}", name=f"wk{tag}")
        wv_sb = per.tile([128, DT, JC], mmdt, tag=f"wv{tag}", name=f"wv{tag}")
        wo_sb = per.tile([128, 4, D], mmdt, tag=f"wo{tag}", name=f"wo{tag}")
        # x^T resident in SBUF, loaded once (bf16 makes this affordable)
        xr = [per.tile([128, DT, SC], mmdt, tag=f"xr{i}{tag}", name=f"xr{i}{tag}")
              for i in range(NSC)]
        # per-chunk Q tiles, double-buffered across q-chunks
        pqt = ctx.enter_context(tc.tile_pool(name=f"pqt{tag}", bufs=2))
        pat = ctx.enter_context(tc.tile_pool(name=f"pat{tag}", bufs=RBUFS))
        pbe = ctx.enter_context(tc.tile_pool(name=f"pbe{tag}", bufs=EBUFS))
        pbt1 = ctx.enter_context(tc.tile_pool(name=f"pbt1{tag}", bufs=1))
        pbt = ctx.enter_context(tc.tile_pool(name=f"pbt{tag}", bufs=2))
        pbo = ctx.enter_context(tc.tile_pool(name=f"pbo{tag}", bufs=OBUFS))
        pbps = ctx.enter_context(tc.tile_pool(name=f"pbps{tag}", bufs=1, space="PSUM"))
        pbps2 = ctx.enter_context(tc.tile_pool(name=f"pbps2{tag}", bufs=2, space="PSUM"))

        xT_r = xT.rearrange("(dt p) s -> p dt s", p=128)

        def load_w(wt, w_ap):
            # one batched DMA: per-slice instructions pay ~400ns of queue
            # overhead each, leaving the DMA engines half idle at startup
            w_r = w_ap.rearrange("(dt p) j -> p dt j", p=128)
            nc.sync.dma_start(out=wt, in_=w_r)

        def rope(ps, dst, ssl, alt=False):
            # RoPE: dst = ps*cos + shuffle(ps)*sin.
            # DVE handles the two PSUM readers (shuffle, cos-mul); GpSimd
            # (SBUF-only) does the sin-mul and the final add -- except when
            # `alt` rebalances the add onto DVE (Pool saturates otherwise
            # when several ropes are queued back-to-back).
            qs = pat.tile([128, SC], F32, tag=f"ropes{tag}", name=f"ropes{tag}")
            qc_t = pat.tile([128, SC], F32, tag=f"ropec{tag}", name=f"ropec{tag}")
            if ROPE_SPLIT and ROPEV == 2:
                nc.vector.stream_shuffle(qs, ps, _PAIR_SWAP)
                nc.vector.tensor_mul(qc_t, ps, cos_sb[:, ssl])
                nc.gpsimd.tensor_mul(qs, qs, sin_sb[:, ssl])
                if alt:
                    nc.vector.tensor_add(dst, qc_t, qs)
                else:
                    nc.gpsimd.tensor_add(dst, qc_t, qs)
            elif ROPE_SPLIT:
                pf = pat.tile([128, SC], F32, tag=f"ropef{tag}", name=f"ropef{tag}")
                nc.vector.tensor_copy(pf, ps)
                nc.vector.stream_shuffle(qs, pf, _PAIR_SWAP)
                nc.vector.tensor_mul(qs, qs, sin_sb[:, ssl])
                nc.gpsimd.tensor_mul(qc_t, pf, cos_sb[:, ssl])
                nc.gpsimd.tensor_add(dst, qc_t, qs)
            else:
                nc.vector.stream_shuffle(qs, ps, _PAIR_SWAP)
                nc.vector.tensor_mul(qs, qs, sin_sb[:, ssl])
                nc.vector.tensor_mul(qc_t, ps, cos_sb[:, ssl])
                nc.vector.tensor_add(dst, qc_t, qs)

        def misc_psum():
            # shares the yps tag: out-proj accumulation and Q/K/V projection
            # accumulation are both sporadic [128, SC] f32 users
            return pbps2.tile([128, SC], F32, tag=f"yps{tag}",
                              name=f"yps{tag}", bufs=YBUFS)

        def proj_q(qc_idx, jt, alt=False):
            ps = misc_psum()
            jl = slice(jt * 128, (jt + 1) * 128)
            for dt in range(DT):
                nc.tensor.matmul(ps, wq_sb[:, dt, jl], xr[qc_idx][:, dt, :],
                                 start=(dt == 0), stop=(dt == DT - 1))
            nqt = pqt.tile([128, SC], mmdt, tag=f"QTq{jt}{tag}",
                           name=f"QTq{jt}{tag}")
            rope(ps, nqt, slice(qc_idx * SC, (qc_idx + 1) * SC), alt=alt)
            return nqt

        def proj_k(sc, jt, alt=False):
            ps = misc_psum()
            jl = slice(jt * 128, (jt + 1) * 128)
            for dt in range(DT):
                nc.tensor.matmul(ps, wk_sb[:, dt, jl], xr[sc][:, dt, :],
                                 start=(dt == 0), stop=(dt == DT - 1))
            ssl = slice(sc * SC, (sc + 1) * SC)
            rope(ps, KTt[jt][:, ssl], ssl, alt=alt)

        def proj_v(sc, st, act=False):
            ps = misc_psum()
            sl = slice(st * 128, (st + 1) * 128)
            for dt in range(DT):
                nc.tensor.matmul(ps, xr[sc][:, dt, sl], wv_sb[:, dt, :],
                                 start=(dt == 0), stop=(dt == DT - 1))
            vt = vo[sc * 4 + st]
            ps_r = ps.rearrange("p (h j) -> p h j", h=HPC)
            if VCOPY_ACT or act:
                nc.scalar.copy(vt[:, :, 0:64], ps_r)
            else:
                nc.vector.tensor_copy(vt[:, :, 0:64], ps_r)

        def emit_outproj(qsl_, oTs_, mts):
            for mt in mts:
                yps = pbps2.tile([128, SC], F32, tag=f"yps{tag}",
                                 name=f"yps{tag}", bufs=YBUFS)
                ml = slice(mt * 128, (mt + 1) * 128)
                for hp_ in range(4):
                    nc.tensor.matmul(yps, wo_sb[:, hp_, ml], oTs_[hp_],
                                     start=(hp_ == 0), stop=(hp_ == 3))
                ys = pbt.tile([128, SC], F32, tag=f"ys{tag}", name=f"ys{tag}")
                nc.vector.tensor_copy(ys, yps)
                nc.sync.dma_start(out=yT[ml, qsl_], in_=ys)

        # ---- DMA issue: weights on the SP queue, x on the Scalar queue
        # (separate queues avoid head-of-line blocking; transfers share the
        # DMA engine pool). wq/x0 first: the init Q0 projection needs them.
        def load_x(i):
            ssl = slice(i * SC, (i + 1) * SC)
            nc.scalar.dma_start(out=xr[i], in_=xT_r[:, :, ssl])

        if DMAORD == 3:
            # first head-pair's wq/wk column slices land first so the init
            # Q0/K0 fills start as soon as x0 arrives
            wqr = wq.rearrange("(dt p) j -> p dt j", p=128)
            wkr = wk.rearrange("(dt p) j -> p dt j", p=128)
            nc.sync.dma_start(out=wq_sb[:, :, 0:128], in_=wqr[:, :, 0:128])
            load_x(0)
            nc.sync.dma_start(out=wk_sb[:, :, 0:128], in_=wkr[:, :, 0:128])
            nc.sync.dma_start(out=cos_sb[:, 0:SC], in_=cosE[:, 0:SC])
            nc.sync.dma_start(out=sin_sb[:, 0:SC], in_=sinE[:, 0:SC])
            load_w(wv_sb, wv)
            nc.sync.dma_start(out=wq_sb[:, :, 128:JC], in_=wqr[:, :, 128:JC])
            nc.sync.dma_start(out=wk_sb[:, :, 128:JC], in_=wkr[:, :, 128:JC])
            nc.sync.dma_start(out=cos_sb[:, SC:], in_=cosE[:, SC:])
            nc.sync.dma_start(out=sin_sb[:, SC:], in_=sinE[:, SC:])
            for i in range(1, NSC):
                load_x(i)
        else:
            load_w(wq_sb, wq)
            load_x(0)
        if DMAORD == 3:
            pass
        elif DMAORD == 0:
            load_w(wk_sb, wk)
            nc.sync.dma_start(out=cos_sb, in_=cosE)
            nc.sync.dma_start(out=sin_sb, in_=sinE)
            for i in range(1, NSC):
                load_x(i)
            load_w(wv_sb, wv)
        elif DMAORD == 2:
            # chunk-0 trig first so K0's RoPE can start early; wv before
            # the x1-3 bulk (V0 is on the init critical path, x1+ are not).
            # The tiny mask comes early too: its device-side cast gates the
            # Act queue, and everything behind it -- including the first exps.
            load_w(wk_sb, wk)
            nc.sync.dma_start(out=cos_sb[:, 0:SC], in_=cosE[:, 0:SC])
            nc.sync.dma_start(out=sin_sb[:, 0:SC], in_=sinE[:, 0:SC])
            nc.sync.dma_start(out=mask_f.rearrange("p a b -> p (a b)"),
                              in_=maskneg)
            load_w(wv_sb, wv)
            nc.sync.dma_start(out=cos_sb[:, SC:], in_=cosE[:, SC:])
            nc.sync.dma_start(out=sin_sb[:, SC:], in_=sinE[:, SC:])
            for i in range(1, NSC):
                load_x(i)
        else:
            nc.sync.dma_start(out=cos_sb, in_=cosE)
            nc.sync.dma_start(out=sin_sb, in_=sinE)
            load_w(wk_sb, wk)
            load_w(wv_sb, wv)
        wo_r = wo.rearrange("(hp p) m -> p hp m", p=128)
        nc.sync.dma_start(out=wo_sb, in_=wo_r)
        # base-0 copy of wo's hp3 rows 64:128: lets the final block's head-B
        # output skip its partition-relocate DMA (matmul bases must match)
        wo3b = per.tile([64, D], mmdt, tag=f"wo3b{tag}", name=f"wo3b{tag}")
        nc.sync.dma_start(out=wo3b, in_=wo_sb[64:128, 3, :])
        if DMAORD != 2:
            nc.sync.dma_start(out=mask_f.rearrange("p a b -> p (a b)"),
                              in_=maskneg)
        nc.vector.memset(ones_sb, 1.0)
        # ones column of V is constant across the whole run: set it once.
        # GpSimd does it (idle at startup) so the Act queue reaches the
        # first exps with only the mask cast ahead of them.
        for i in range(NKT):
            nc.gpsimd.tensor_copy(vo[i][:, :, 64:65],
                                  ones_sb.rearrange("p (h o) -> p h o", o=1))

        # ---- init: just enough of chunk 0 for the first hp block; the
        # rest of chunk 0's Q/K streams through qc0's hp loop below.
        cur_QT = [None] * 4
        if INIT_STREAM:
            cur_QT[0] = proj_q(0, 0)
            proj_k(0, 0)
        else:
            ia = IROPE >= 1
            cur_QT[0] = proj_q(0, 0, alt=False)
            # one extra Q fill bridges PE's wait for the wk DMA before the
            # first K fill can start
            cur_QT[1] = proj_q(0, 1, alt=False)
            proj_k(0, 0, alt=ia)
            for jt in range(2, 4):
                cur_QT[jt] = proj_q(0, jt, alt=ia and jt % 2 == 1)
            for jt in range(1, 4):
                proj_k(0, jt, alt=ia and jt % 2 == 0)
        # V0 tiles are woven into the first hp block's kt loop (emitted just
        # before the PV matmul that first needs each one) so the first
        # scores/exp are not queued behind four V fills on the in-order PE.
        # mask cast on GpSimd after the init ropes: keeps the Act queue
        # free of everything but exps; its first consumer is much later
        nc.gpsimd.tensor_copy(mask_sb, mask_f)
        if DMAORD != 0:
            load_x(1)  # x chunks 2/3 are issued lazily at later qc tops

        # ---- main loop: attention(qc) + projections(qc+1) interleaved ---
        prev = None  # (qsl, oTs) of the previous q-chunk; its out-proj is
        # deferred into this chunk's hp loop so ready matmuls fill PE.
        final_pre = []  # prefilled yps tiles of the last chunk's out-proj
        last_tmpB = [None]  # final block's head-B output (base 0)
        for qc in range(NSC):
            qsl = slice(qc * SC, (qc + 1) * SC)
            if DMAORD != 0 and qc + 2 < NSC:
                load_x(qc + 2)
            next_QT = [None] * 4
            oTs = []
            for hp in range(4):
                QTh = cur_QT[hp]
                pva = pbps.tile([65, SC], F32, tag=f"pva{tag}", name=f"pva{tag}")
                pvb = pbps.tile([65, SC], F32, tag=f"pvb{tag}", name=f"pvb{tag}")
                nkt = 4 * qc + 4
                h0, h1 = 2 * hp, 2 * hp + 1
                pending = None  # software pipeline: PV lags scores by 1
                for kt in range(nkt):
                    ksl = slice(kt * KT, (kt + 1) * KT)
                    d = kt - 4 * qc
                    # diagonal tiles: only columns q >= 128*d are causally
                    # valid -- shrink scores/exp/PV to that range; the
                    # boundary 128-wide strip still needs the triangular
                    # mask.
                    cs = 128 * d if d > 0 else 0
                    vq = slice(cs, SC)
                    sc2 = pbps2.tile([128, 2, SC], F32, tag=f"sc2{tag}",
                                     name=f"sc2{tag}", bufs=SCBUFS)
                    sca, scb = sc2[:, 0, :], sc2[:, 1, :]
                    nc.tensor.matmul(sca[:, vq], KTt[hp][0:64, ksl],
                                     QTh[0:64, vq],
                                     start=True, stop=True,
                                     tile_position=(0, 0))
                    nc.tensor.matmul(scb[:, vq], KTt[hp][64:128, ksl],
                                     QTh[64:128, vq],
                                     start=True, stop=True,
                                     tile_position=(64, 0))
                    e2 = pbe.tile([128, 2, SC], pvdt, tag=f"e2{tag}",
                                  name=f"e2{tag}")
                    nc.scalar.activation(e2[:, :, vq], sc2[:, :, vq],
                                         EXP, scale=0.125)
                    if d >= 0:
                        # causal boundary strip: zero the exp of the
                        # invalid (q < k) positions with a 0/1 mask.
                        # GpSimd can do this since e2 lives in SBUF.
                        bs = slice(cs, cs + 128)
                        if MASK_GPS:
                            nc.gpsimd.tensor_mul(e2[:, :, bs],
                                                 e2[:, :, bs], mask_sb)
                        else:
                            nc.vector.tensor_mul(e2[:, :, bs],
                                                 e2[:, :, bs], mask_sb)
                    ea, eb = e2[:, 0, :], e2[:, 1, :]
                    if qc == 0 and hp == 0 and not INIT_STREAM:
                        # weave V0 fills ahead of their PV use
                        proj_v(0, kt, act=VACT >= 1)
                    if pending is not None:
                        pkt, pea, peb, pvq = pending
                        nc.tensor.matmul(pva[:, pvq], vo[pkt][:, h0, :],
                                         pea[:, pvq],
                                         start=(pkt == 0), stop=False)
                        nc.tensor.matmul(pvb[:, pvq], vo[pkt][:, h1, :],
                                         peb[:, pvq],
                                         start=(pkt == 0), stop=False)
                    pending = (kt, ea, eb, vq)
                pkt, pea, peb, pvq = pending
                nc.tensor.matmul(pva[:, pvq], vo[pkt][:, h0, :],
                                 pea[:, pvq],
                                 start=(pkt == 0), stop=True)
                nc.tensor.matmul(pvb[:, pvq], vo[pkt][:, h1, :],
                                 peb[:, pvq],
                                 start=(pkt == 0), stop=True)
                def ready_work(hp=hp):
                    # chunk-0 stragglers, the previous chunk's out-proj, and
                    # chunk qc+1's projections -- all runnable immediately.
                    if qc == 0 and hp + 1 < 4 and INIT_STREAM:
                        cur_QT[hp + 1] = proj_q(0, hp + 1)
                        proj_k(0, hp + 1)
                    if prev is not None:
                        counts = {0: [2, 2, 2, 2], 1: [4, 4, 0, 0],
                                  2: [3, 3, 2, 0]}[OPDIST]
                        lo = sum(counts[:hp])
                        mts = range(lo, lo + counts[hp])
                        emit_outproj(prev[0], prev[1], mts)
                    if qc + 1 < NSC:
                        next_QT[hp] = proj_q(qc + 1, hp, alt=IROPE == 2)
                        proj_k(qc + 1, hp)
                        proj_v(qc + 1, hp, act=qc < VACT)
                    elif hp == 3:
                        # last block: prefill the final out-proj's first two
                        # column tiles over hp0-2 while hp3 still computes,
                        # so only the +hp3 matmul remains after its oT lands
                        for mt in (0, 1):
                            yps = pbps2.tile([128, SC], F32, tag=f"yps{tag}",
                                             name=f"yps{tag}", bufs=YBUFS)
                            ml = slice(mt * 128, (mt + 1) * 128)
                            for hp_ in range(3):
                                nc.tensor.matmul(yps, wo_sb[:, hp_, ml],
                                                 oTs[hp_], start=(hp_ == 0),
                                                 stop=False)
                            final_pre.append((mt, yps))

                if BLKORD == 1:
                    ready_work()
                # normalize: oT[j, q] = pv[j, q] / denom[q].
                # Stage PSUM -> SBUF (partition-aligned) so pva/pvb free
                # early; reciprocal reads the PSUM denom row directly
                # (in@p64 -> out@p0 is valid for single-input DVE ops);
                # broadcast to 64 partitions at base 0; head B's rows are
                # DMA-relocated to 64:128 (engine ops cannot cross-base).
                last_blk = qc == NSC - 1 and hp == 3
                d2 = pbt1.tile([1, 2, SC], F32, tag=f"d2{tag}", name=f"d2{tag}")
                # reciprocals first: they gate the broadcast+mul chain, the
                # copies don't
                nc.vector.reciprocal(d2[:, 0, :], pva[64:65, :])
                nc.vector.reciprocal(d2[:, 1, :], pvb[64:65, :])
                if last_blk:
                    # nothing needs the PSUM banks after this block: skip
                    # the staging copies, multiply straight from PSUM
                    o2a, o2b = pva[0:64, :], pvb[0:64, :]
                else:
                    o2 = pbt1.tile([65, 2, SC], F32, tag=f"o2{tag}", name=f"o2{tag}")
                    if qc < O2ACT:
                        # ScalarE has slack in the early chunks; late chunks
                        # are exp-bound so the copies go to DVE there.
                        nc.scalar.copy(o2[:, 0, :], pva)
                        nc.scalar.copy(o2[:, 1, :], pvb)
                    else:
                        nc.vector.tensor_copy(o2[:, 0, :], pva)
                        nc.vector.tensor_copy(o2[:, 1, :], pvb)
                    o2a, o2b = o2[0:64, 0, :], o2[0:64, 1, :]
                bc = pbt1.tile([64, 2, SC], F32, tag=f"bc{tag}", name=f"bc{tag}")
                nc.gpsimd.partition_broadcast(bc[:, 0, :], d2[:, 0, :])
                nc.gpsimd.partition_broadcast(bc[:, 1, :], d2[:, 1, :])
                oT = pbo.tile([128, SC], mmdt, tag=f"oT{hp}{tag}", name=f"oT{hp}{tag}")
                tmpB = pbt.tile([64, SC], mmdt, tag=f"tmpB{tag}", name=f"tmpB{tag}")
                nc.vector.tensor_mul(oT[0:64, :], o2a, bc[:, 0, :])
                nc.vector.tensor_mul(tmpB, o2b, bc[:, 1, :])
                if last_blk:
                    # head B stays at base 0; the final out-proj contracts
                    # it against wo3b instead of waiting for a relocate
                    last_tmpB[0] = tmpB
                else:
                    nc.sync.dma_start(out=oT[64:128, :], in_=tmpB)
                oTs.append(oT)
                if BLKORD == 0:
                    ready_work()
            prev = (qsl, oTs)
            if qc + 1 < NSC:
                cur_QT = next_QT
        # final out-proj: finish the prefilled tiles (+hp3 only), then the
        # rest; alternate ys staging between DVE and ScalarE so the last
        # copies drain in parallel
        qsl_, oTs_ = prev
        done_pre = {mt for mt, _ in final_pre}
        seq = [(mt, yps) for mt, yps in final_pre]
        seq += [(mt, None) for mt in range(8) if mt not in done_pre]
        for i, (mt, yps) in enumerate(seq):
            ml = slice(mt * 128, (mt + 1) * 128)
            if yps is None:
                yps = pbps2.tile([128, SC], F32, tag=f"yps{tag}",
                                 name=f"yps{tag}", bufs=YBUFS)
                for hp_ in range(3):
                    nc.tensor.matmul(yps, wo_sb[:, hp_, ml], oTs_[hp_],
                                     start=(hp_ == 0), stop=False)
            # hp3 contribution in two base-0 halves (no relocate needed)
            nc.tensor.matmul(yps, wo_sb[0:64, 3, ml], oTs_[3][0:64, :],
                             start=False, stop=False)
            nc.tensor.matmul(yps, wo3b[:, ml], last_tmpB[0],
                             start=False, stop=True)
            ys = pbt.tile([128, SC], F32, tag=f"ys{tag}", name=f"ys{tag}")
            if i % 2 == 0:
                nc.vector.tensor_copy(ys, yps)
            else:
                nc.scalar.copy(ys, yps)
            nc.sync.dma_start(out=yT[ml, qsl_], in_=ys)


_BUILT = {}


def build_nc(mmdt_name=MM_DTYPE, repeat=1):
    key = (mmdt_name, repeat)
    if key in _BUILT:
        return _BUILT[key]
    mmdt = {"f32": F32, "f32r": F32R, "bf16": BF16}[mmdt_name]
    nc = bacc.Bacc("TRN2", target_bir_lowering=False, debug=False,
                   num_devices=N_CORES)
    dram = {
        "xT": nc.dram_tensor("xT", [D, S], mmdt, kind="ExternalInput").ap(),
        "wq": nc.dram_tensor("wq", [D, JC], mmdt, kind="ExternalInput").ap(),
        "wk": nc.dram_tensor("wk", [D, JC], mmdt, kind="ExternalInput").ap(),
        "wv": nc.dram_tensor("wv", [D, JC], mmdt, kind="ExternalInput").ap(),
        "wo": nc.dram_tensor("wo", [JC, D], mmdt, kind="ExternalInput").ap(),
        "cosE": nc.dram_tensor("cosE", [128, S], F32,
                               kind="ExternalInput").ap(),
        "sinE": nc.dram_tensor("sinE", [128, S], F32,
                               kind="ExternalInput").ap(),
        "maskneg": nc.dram_tensor("maskneg", [128, 256], F32,
                                  kind="ExternalInput").ap(),
        "yT": nc.dram_tensor("yT", [D, S], F32, kind="ExternalOutput").ap(),
    }
    with tile.TileContext(nc) as tc:
        for r in range(repeat):
            _emit(nc, tc, mmdt, dram, tag=f"r{r}" if repeat > 1 else "")
    nc.compile()
    _BUILT[key] = nc
    return nc


def _round_f32r(a):
    """Round-to-nearest onto the f32r grid (fp32 with low 12 mantissa bits 0)."""
    b = np.ascontiguousarray(a, np.float32).view(np.uint32).astype(np.uint64)
    b = (b + 0x800 + ((b >> 12) & 1)) & 0xFFFFF000
    return b.astype(np.uint32).view(np.float32)


def _to_bf16(a):
    import ml_dtypes
    return np.ascontiguousarray(a, np.float32).astype(ml_dtypes.bfloat16)


def _host_prep(x, pos_ids, Wq, Wk, Wv, Wo, cos, sin, mmdt_name=None):
    """Build the 8 per-core input maps."""
    if mmdt_name is None:
        mmdt_name = MM_DTYPE
    if mmdt_name == "f32r":
        rnd = _round_f32r
    elif mmdt_name == "bf16":
        rnd = _to_bf16
    else:
        rnd = lambda a: a
    x = np.asarray(x, dtype=np.float32)
    pos_ids = np.asarray(pos_ids)
    cos = np.asarray(cos, dtype=np.float32)
    sin = np.asarray(sin, dtype=np.float32)
    freq_idx = np.tile(np.repeat(np.arange(DK // 2), 2), 2)  # [128]
    sign = np.where((np.arange(128) % 2) == 0, -1.0, 1.0).astype(np.float32)

    # universal triangular boundary mask: 1 if q >= p else 0 (multiplied
    # into exp(scores) post-activation); two side-by-side copies (one per
    # head in the merged [128, 2, 128] tile)
    p = np.arange(128)[:, None]
    q = np.arange(128)[None, :]
    mask1 = np.where(q >= p, 1.0, 0.0).astype(np.float32)
    mask = np.concatenate([mask1, mask1], axis=1)  # [128, 256]

    in_maps = []
    for c in range(N_CORES):
        b, g = c // 2, c % 2
        hs = slice(64 * HPC * g, 64 * HPC * g + JC)
        pos = pos_ids[b].astype(np.int64)
        cosT = cos[pos].T  # [32, S]
        sinT = sin[pos].T
        cosE = np.ascontiguousarray(cosT[freq_idx])           # [128, S]
        sinE = np.ascontiguousarray(sinT[freq_idx] * sign[:, None])
        in_maps.append({
            "xT": rnd(np.ascontiguousarray(x[b].T)),
            "wq": rnd(np.ascontiguousarray(Wq[hs, :].T)),
            "wk": rnd(np.ascontiguousarray(Wk[hs, :].T)),
            "wv": rnd(np.ascontiguousarray(Wv[hs, :].T)),
            "wo": rnd(np.ascontiguousarray(Wo[:, hs].T)),
            "cosE": cosE,
            "sinE": sinE,
            "maskneg": mask,
        })
    return in_maps


def kernel(x, pos_ids, Wq, Wk, Wv, Wo, cos, sin):
    nc = build_nc()
    in_maps = _host_prep(x, pos_ids, Wq, Wk, Wv, Wo, cos, sin)
    res = run_bass_kernel_spmd(nc, in_maps, list(range(N_CORES)))
    out = np.empty((B, S, D), dtype=np.float32)
    for b in range(B):
        yT = res.results[2 * b]["yT"] + res.results[2 * b + 1]["yT"]
        out[b] = yT.T
    return out


if __name__ == "__main__":
    t0 = time.time()
    nc = build_nc()
    print(f"build+compile: {time.time()-t0:.1f}s", flush=True)


# revision 97
# speedup vs baseline: 1.0056x; 1.0056x over previous
"""Causal multi-head self-attention with RoPE on 8 TRN2 NeuronCores.

Sharding: batch(4) x head-group(2) -> 8 cores. Core c handles batch c//2 and
heads [8*(c%2), 8*(c%2)+8). Each core computes its partial output projection
(sum over its 8 heads' contribution); the host adds the two head-group
partials per batch. No device collectives needed.

Fully streamed single-pass schedule: after a small init (Q/K/V of chunk 0),
chunk qc's attention runs while chunk qc+1's Q/K/V projections are emitted
between its head-pair blocks -- causality guarantees attention(qc) only
needs chunks <= qc. The output projection of chunk qc-1 is likewise deferred
into chunk qc's loop so ready matmuls keep PE fed while exp/normalize drain.

Dtypes: matmul operands are bf16 (x resident in SBUF, loaded once; weights,
Q/K/V, P=exp(scores), output proj) -- on the PE this costs the same
cycles/row as f32r but halves SBUF/DMA and avoids the f32r narrow-tile
penalty. All accumulation is f32 in PSUM; softmax normalize runs in f32;
the final y is f32. Measured rel err vs the f32 reference ~= 4e-3.

On-chip layout: sequence on the free dimension everywhere.
  - Q^T/K^T [j, s] from projection matmuls (lhsT = W^T slices, rhs = x^T),
    RoPE via partition pair-swap (stream_shuffle) + cos/sin tables; the
    elementwise work is split DVE (shuffle/cos-mul, the PSUM readers) /
    GpSimd (sin-mul/add, SBUF only).
  - scores S^T [k, q] per k-tile into a merged [128, 2, SC] PSUM tile (both
    heads of the pair), one exp per k-tile on ScalarE (|scores|/8 <= ~15 so
    no max subtraction), causal boundary strips zeroed post-exp with a 0/1
    mask on DVE.
  - P@V contracts k on partitions; a ones-row appended to V yields the
    softmax denominator from the same matmul (M=65).
  - output projection contracts the 512 head-dims -> partial y^T [1024, s].
"""

import os
import sys
import time

for _p in ("/opt/trn_rl_repo", "/root/.axon_site/_ro/trn_rl_repo"):
    if _p not in sys.path and os.path.isdir(_p):
        sys.path.insert(0, _p)

import numpy as np
import concourse.bass as bass
import concourse.bacc as bacc
import concourse.mybir as mybir
import concourse.tile as tile
from concourse.bass_utils import run_bass_kernel_spmd

F32 = mybir.dt.float32
F32R = mybir.dt.float32r
BF16 = mybir.dt.bfloat16

B, S, D = 4, 2048, 1024
H, DK = 16, 64
HPC = 8            # heads per core
JC = HPC * DK      # 512 head-dims per core
N_CORES = 8
SC = 512           # q-chunk width (moving free dim)
NSC = S // SC      # 4
KT = 128           # k-tile (scores partition dim)
NKT = S // KT      # 16
DT = D // 128      # 8 contraction tiles for projections

# matmul operand dtype: "bf16" (fast, rel err ~4e-3), "f32r", or "f32"
MM_DTYPE = os.environ.get("KERNEL_MM_DTYPE", "bf16")
EBUFS = int(os.environ.get("KV_EBUFS", "17"))
SCBUFS = int(os.environ.get("KV_SCBUFS", "2"))
YBUFS = int(os.environ.get("KV_YBUFS", "2"))
XBUFS = int(os.environ.get("KV_XBUFS", "2"))
RBUFS = int(os.environ.get("KV_RBUFS", "5"))
OBUFS = int(os.environ.get("KV_OBUFS", "2"))
ROPE_SPLIT = os.environ.get("KV_ROPE_SPLIT", "1") == "1"
VCOPY_ACT = os.environ.get("KV_VCOPY_ACT", "0") == "1"
MASK_GPS = os.environ.get("KV_MASK_GPS", "0") == "1"
BLKORD = int(os.environ.get("KV_BLKORD", "1"))    # 1: ready-work pre-normalize
ROPEV = int(os.environ.get("KV_ROPEV", "2"))      # 1: pf staging, 2: direct
O2ACT = int(os.environ.get("KV_O2ACT", "3"))      # o2 copies on Act for qc<N
INIT_STREAM = os.environ.get("KV_INIT_STREAM", "0") == "1"
DMAORD = int(os.environ.get("KV_DMAORD", "2"))    # 2: chunk-0 trig early
IROPE = int(os.environ.get("KV_IROPE", "0"))      # rope add alternates to DVE:
                                                  # 0 never, 1 init, 2 always
OPDIST = int(os.environ.get("KV_OPDIST", "0"))    # deferred out-proj mts per
                                                  # hp block: 0 = [2,2,2,2],
                                                  # 1 = [4,4,0,0], 2 = [3,3,2,0]
VACT = int(os.environ.get("KV_VACT", "0"))        # V copies on Act for qc<N
                                                  # (DVE is rope-bound early)
YBF16 = os.environ.get("KV_YBF16", "1") == "1"    # y output in bf16 (host
                                                  # upconverts before the add)


_PAIR_SWAP = []
for _i in range(16):
    _PAIR_SWAP += [2 * _i + 1, 2 * _i]


def _emit(nc, tc, mmdt, dram, tag=""):
    """Emit the whole per-core program. `dram` maps name -> DRAM AP."""
    xT = dram["xT"]
    wq, wk, wv, wo = dram["wq"], dram["wk"], dram["wv"], dram["wo"]
    cosE, sinE, maskneg = dram["cosE"], dram["sinE"], dram["maskneg"]
    yT = dram["yT"]

    pvdt = mmdt
    EXP = mybir.ActivationFunctionType.Exp

    import contextlib
    with contextlib.ExitStack() as ctx:
        # ---- persistent tiles -------------------------------------------
        per = ctx.enter_context(tc.tile_pool(name=f"per{tag}", bufs=1))
        KTt = [per.tile([128, S], mmdt, tag=f"KT{j}{tag}", name=f"KT{j}{tag}") for j in range(4)]
        vo = [per.tile([128, HPC, 65], pvdt, tag=f"vo{i}{tag}", name=f"vo{i}{tag}") for i in range(NKT)]
        ones_sb = per.tile([128, HPC], F32, tag=f"ones{tag}", name=f"ones{tag}")
        cos_sb = per.tile([128, S], F32, tag=f"cos{tag}", name=f"cos{tag}")
        sin_sb = per.tile([128, S], F32, tag=f"sin{tag}", name=f"sin{tag}")
        mask_f = per.tile([128, 2, 128], F32, tag=f"maskf{tag}", name=f"maskf{tag}")
        mask_sb = per.tile([128, 2, 128], pvdt, tag=f"mask{tag}", name=f"mask{tag}")
        wq_sb = per.tile([128, DT, JC], mmdt, tag=f"wq{tag}", name=f"wq{tag}")
        wk_sb = per.tile([128, DT, JC], mmdt, tag=f"wk{tag}", name=f"wk{tag}")
        wv_sb = per.tile([128, DT, JC], mmdt, tag=f"wv{tag}", name=f"wv{tag}")
        wo_sb = per.tile([128, 4, D], mmdt, tag=f"wo{tag}", name=f"wo{tag}")
        # x^T resident in SBUF, loaded once (bf16 makes this affordable)
        xr = [per.tile([128, DT, SC], mmdt, tag=f"xr{i}{tag}", name=f"xr{i}{tag}")
              for i in range(NSC)]
        # per-chunk Q tiles, double-buffered across q-chunks
        pqt = ctx.enter_context(tc.tile_pool(name=f"pqt{tag}", bufs=2))
        pat = ctx.enter_context(tc.tile_pool(name=f"pat{tag}", bufs=RBUFS))
        pbe = ctx.enter_context(tc.tile_pool(name=f"pbe{tag}", bufs=EBUFS))
        pbt1 = ctx.enter_context(tc.tile_pool(name=f"pbt1{tag}", bufs=1))
        pbt = ctx.enter_context(tc.tile_pool(name=f"pbt{tag}", bufs=2))
        pbo = ctx.enter_context(tc.tile_pool(name=f"pbo{tag}", bufs=OBUFS))
        pbps = ctx.enter_context(tc.tile_pool(name=f"pbps{tag}", bufs=1, space="PSUM"))
        pbps2 = ctx.enter_context(tc.tile_pool(name=f"pbps2{tag}", bufs=2, space="PSUM"))

        xT_r = xT.rearrange("(dt p) s -> p dt s", p=128)

        def load_w(wt, w_ap):
            # one batched DMA: per-slice instructions pay ~400ns of queue
            # overhead each, leaving the DMA engines half idle at startup
            w_r = w_ap.rearrange("(dt p) j -> p dt j", p=128)
            nc.sync.dma_start(out=wt, in_=w_r)

        def rope(ps, dst, ssl, alt=False):
            # RoPE: dst = ps*cos + shuffle(ps)*sin.
            # DVE handles the two PSUM readers (shuffle, cos-mul); GpSimd
            # (SBUF-only) does the sin-mul and the final add -- except when
            # `alt` rebalances the add onto DVE (Pool saturates otherwise
            # when several ropes are queued back-to-back).
            qs = pat.tile([128, SC], F32, tag=f"ropes{tag}", name=f"ropes{tag}")
            qc_t = pat.tile([128, SC], F32, tag=f"ropec{tag}", name=f"ropec{tag}")
            if ROPE_SPLIT and ROPEV == 2:
                nc.vector.stream_shuffle(qs, ps, _PAIR_SWAP)
                nc.vector.tensor_mul(qc_t, ps, cos_sb[:, ssl])
                nc.gpsimd.tensor_mul(qs, qs, sin_sb[:, ssl])
                if alt:
                    nc.vector.tensor_add(dst, qc_t, qs)
                else:
                    nc.gpsimd.tensor_add(dst, qc_t, qs)
            elif ROPE_SPLIT:
                pf = pat.tile([128, SC], F32, tag=f"ropef{tag}", name=f"ropef{tag}")
                nc.vector.tensor_copy(pf, ps)
                nc.vector.stream_shuffle(qs, pf, _PAIR_SWAP)
                nc.vector.tensor_mul(qs, qs, sin_sb[:, ssl])
                nc.gpsimd.tensor_mul(qc_t, pf, cos_sb[:, ssl])
                nc.gpsimd.tensor_add(dst, qc_t, qs)
            else:
                nc.vector.stream_shuffle(qs, ps, _PAIR_SWAP)
                nc.vector.tensor_mul(qs, qs, sin_sb[:, ssl])
                nc.vector.tensor_mul(qc_t, ps, cos_sb[:, ssl])
                nc.vector.tensor_add(dst, qc_t, qs)

        def misc_psum():
            # shares the yps tag: out-proj accumulation and Q/K/V projection
            # accumulation are both sporadic [128, SC] f32 users
            return pbps2.tile([128, SC], F32, tag=f"yps{tag}",
                              name=f"yps{tag}", bufs=YBUFS)

        def proj_q(qc_idx, jt, alt=False):
            ps = misc_psum()
            jl = slice(jt * 128, (jt + 1) * 128)
            for dt in range(DT):
                nc.tensor.matmul(ps, wq_sb[:, dt, jl], xr[qc_idx][:, dt, :],
                                 start=(dt == 0), stop=(dt == DT - 1))
            nqt = pqt.tile([128, SC], mmdt, tag=f"QTq{jt}{tag}",
                           name=f"QTq{jt}{tag}")
            rope(ps, nqt, slice(qc_idx * SC, (qc_idx + 1) * SC), alt=alt)
            return nqt

        def proj_k(sc, jt, alt=False):
            ps = misc_psum()
            jl = slice(jt * 128, (jt + 1) * 128)
            for dt in range(DT):
                nc.tensor.matmul(ps, wk_sb[:, dt, jl], xr[sc][:, dt, :],
                                 start=(dt == 0), stop=(dt == DT - 1))
            ssl = slice(sc * SC, (sc + 1) * SC)
            rope(ps, KTt[jt][:, ssl], ssl, alt=alt)

        def proj_v(sc, st, act=False):
            ps = misc_psum()
            sl = slice(st * 128, (st + 1) * 128)
            for dt in range(DT):
                nc.tensor.matmul(ps, xr[sc][:, dt, sl], wv_sb[:, dt, :],
                                 start=(dt == 0), stop=(dt == DT - 1))
            vt = vo[sc * 4 + st]
            ps_r = ps.rearrange("p (h j) -> p h j", h=HPC)
            if VCOPY_ACT or act:
                nc.scalar.copy(vt[:, :, 0:64], ps_r)
            else:
                nc.vector.tensor_copy(vt[:, :, 0:64], ps_r)

        def emit_outproj(qsl_, oTs_, mts):
            for mt in mts:
                yps = pbps2.tile([128, SC], F32, tag=f"yps{tag}",
                                 name=f"yps{tag}", bufs=YBUFS)
                ml = slice(mt * 128, (mt + 1) * 128)
                for hp_ in range(4):
                    nc.tensor.matmul(yps, wo_sb[:, hp_, ml], oTs_[hp_],
                                     start=(hp_ == 0), stop=(hp_ == 3))
                ydt = BF16 if YBF16 else F32
                ys = pbt.tile([128, SC], ydt, tag=f"ys{tag}", name=f"ys{tag}")
                nc.vector.tensor_copy(ys, yps)
                nc.sync.dma_start(out=yT[ml, qsl_], in_=ys)

        # ---- DMA issue: weights on the SP queue, x on the Scalar queue
        # (separate queues avoid head-of-line blocking; transfers share the
        # DMA engine pool). wq/x0 first: the init Q0 projection needs them.
        def load_x(i):
            ssl = slice(i * SC, (i + 1) * SC)
            nc.scalar.dma_start(out=xr[i], in_=xT_r[:, :, ssl])

        if DMAORD == 3:
            # first head-pair's wq/wk column slices land first so the init
            # Q0/K0 fills start as soon as x0 arrives
            wqr = wq.rearrange("(dt p) j -> p dt j", p=128)
            wkr = wk.rearrange("(dt p) j -> p dt j", p=128)
            nc.sync.dma_start(out=wq_sb[:, :, 0:128], in_=wqr[:, :, 0:128])
            load_x(0)
            nc.sync.dma_start(out=wk_sb[:, :, 0:128], in_=wkr[:, :, 0:128])
            nc.sync.dma_start(out=cos_sb[:, 0:SC], in_=cosE[:, 0:SC])
            nc.sync.dma_start(out=sin_sb[:, 0:SC], in_=sinE[:, 0:SC])
            load_w(wv_sb, wv)
            nc.sync.dma_start(out=wq_sb[:, :, 128:JC], in_=wqr[:, :, 128:JC])
            nc.sync.dma_start(out=wk_sb[:, :, 128:JC], in_=wkr[:, :, 128:JC])
            nc.sync.dma_start(out=cos_sb[:, SC:], in_=cosE[:, SC:])
            nc.sync.dma_start(out=sin_sb[:, SC:], in_=sinE[:, SC:])
            for i in range(1, NSC):
                load_x(i)
        else:
            load_w(wq_sb, wq)
            load_x(0)
        if DMAORD == 3:
            pass
        elif DMAORD == 0:
            load_w(wk_sb, wk)
            nc.sync.dma_start(out=cos_sb, in_=cosE)
            nc.sync.dma_start(out=sin_sb, in_=sinE)
            for i in range(1, NSC):
                load_x(i)
            load_w(wv_sb, wv)
        elif DMAORD == 2:
            # chunk-0 trig first so K0's RoPE can start early; wv before
            # the x1-3 bulk (V0 is on the init critical path, x1+ are not).
            # The tiny mask comes early too: its device-side cast gates the
            # Act queue, and everything behind it -- including the first exps.
            load_w(wk_sb, wk)
            nc.sync.dma_start(out=cos_sb[:, 0:SC], in_=cosE[:, 0:SC])
            nc.sync.dma_start(out=sin_sb[:, 0:SC], in_=sinE[:, 0:SC])
            nc.sync.dma_start(out=mask_f.rearrange("p a b -> p (a b)"),
                              in_=maskneg)
            load_w(wv_sb, wv)
            nc.sync.dma_start(out=cos_sb[:, SC:], in_=cosE[:, SC:])
            nc.sync.dma_start(out=sin_sb[:, SC:], in_=sinE[:, SC:])
            for i in range(1, NSC):
                load_x(i)
        else:
            nc.sync.dma_start(out=cos_sb, in_=cosE)
            nc.sync.dma_start(out=sin_sb, in_=sinE)
            load_w(wk_sb, wk)
            load_w(wv_sb, wv)
        wo_r = wo.rearrange("(hp p) m -> p hp m", p=128)
        nc.sync.dma_start(out=wo_sb, in_=wo_r)
        # base-0 copy of wo's hp3 rows 64:128: lets the final block's head-B
        # output skip its partition-relocate DMA (matmul bases must match)
        wo3b = per.tile([64, D], mmdt, tag=f"wo3b{tag}", name=f"wo3b{tag}")
        nc.sync.dma_start(out=wo3b, in_=wo_sb[64:128, 3, :])
        if DMAORD != 2:
            nc.sync.dma_start(out=mask_f.rearrange("p a b -> p (a b)"),
                              in_=maskneg)
        nc.vector.memset(ones_sb, 1.0)
        # ones column of V is constant across the whole run: set it once.
        # GpSimd does it (idle at startup) so the Act queue reaches the
        # first exps with only the mask cast ahead of them.
        for i in range(NKT):
            nc.gpsimd.tensor_copy(vo[i][:, :, 64:65],
                                  ones_sb.rearrange("p (h o) -> p h o", o=1))

        # ---- init: just enough of chunk 0 for the first hp block; the
        # rest of chunk 0's Q/K streams through qc0's hp loop below.
        cur_QT = [None] * 4
        if INIT_STREAM:
            cur_QT[0] = proj_q(0, 0)
            proj_k(0, 0)
        else:
            ia = IROPE >= 1
            cur_QT[0] = proj_q(0, 0, alt=False)
            # one extra Q fill bridges PE's wait for the wk DMA before the
            # first K fill can start
            cur_QT[1] = proj_q(0, 1, alt=False)
            proj_k(0, 0, alt=ia)
            for jt in range(2, 4):
                cur_QT[jt] = proj_q(0, jt, alt=ia and jt % 2 == 1)
            for jt in range(1, 4):
                proj_k(0, jt, alt=ia and jt % 2 == 0)
        # V0 tiles are woven into the first hp block's kt loop (emitted just
        # before the PV matmul that first needs each one) so the first
        # scores/exp are not queued behind four V fills on the in-order PE.
        # mask cast on GpSimd after the init ropes: keeps the Act queue
        # free of everything but exps; its first consumer is much later
        nc.gpsimd.tensor_copy(mask_sb, mask_f)
        if DMAORD != 0:
            load_x(1)  # x chunks 2/3 are issued lazily at later qc tops

        # ---- main loop: attention(qc) + projections(qc+1) interleaved ---
        prev = None  # (qsl, oTs) of the previous q-chunk; its out-proj is
        # deferred into this chunk's hp loop so ready matmuls fill PE.
        final_pre = []  # prefilled yps tiles of the last chunk's out-proj
        last_tmpB = [None]  # final block's head-B output (base 0)
        for qc in range(NSC):
            qsl = slice(qc * SC, (qc + 1) * SC)
            if DMAORD != 0 and qc + 2 < NSC:
                load_x(qc + 2)
            next_QT = [None] * 4
            oTs = []
            for hp in range(4):
                QTh = cur_QT[hp]
                pva = pbps.tile([65, SC], F32, tag=f"pva{tag}", name=f"pva{tag}")
                pvb = pbps.tile([65, SC], F32, tag=f"pvb{tag}", name=f"pvb{tag}")
                nkt = 4 * qc + 4
                h0, h1 = 2 * hp, 2 * hp + 1
                pending = None  # software pipeline: PV lags scores by 1
                for kt in range(nkt):
                    ksl = slice(kt * KT, (kt + 1) * KT)
                    d = kt - 4 * qc
                    # diagonal tiles: only columns q >= 128*d are causally
                    # valid -- shrink scores/exp/PV to that range; the
                    # boundary 128-wide strip still needs the triangular
                    # mask.
                    cs = 128 * d if d > 0 else 0
                    vq = slice(cs, SC)
                    sc2 = pbps2.tile([128, 2, SC], F32, tag=f"sc2{tag}",
                                     name=f"sc2{tag}", bufs=SCBUFS)
                    sca, scb = sc2[:, 0, :], sc2[:, 1, :]
                    nc.tensor.matmul(sca[:, vq], KTt[hp][0:64, ksl],
                                     QTh[0:64, vq],
                                     start=True, stop=True,
                                     tile_position=(0, 0))
                    nc.tensor.matmul(scb[:, vq], KTt[hp][64:128, ksl],
                                     QTh[64:128, vq],
                                     start=True, stop=True,
                                     tile_position=(64, 0))
                    e2 = pbe.tile([128, 2, SC], pvdt, tag=f"e2{tag}",
                                  name=f"e2{tag}")
                    nc.scalar.activation(e2[:, :, vq], sc2[:, :, vq],
                                         EXP, scale=0.125)
                    if d >= 0:
                        # causal boundary strip: zero the exp of the
                        # invalid (q < k) positions with a 0/1 mask.
                        # GpSimd can do this since e2 lives in SBUF.
                        bs = slice(cs, cs + 128)
                        if MASK_GPS:
                            nc.gpsimd.tensor_mul(e2[:, :, bs],
                                                 e2[:, :, bs], mask_sb)
                        else:
                            nc.vector.tensor_mul(e2[:, :, bs],
                                                 e2[:, :, bs], mask_sb)
                    ea, eb = e2[:, 0, :], e2[:, 1, :]
                    if qc == 0 and hp == 0 and not INIT_STREAM:
                        # weave V0 fills ahead of their PV use
                        proj_v(0, kt, act=VACT >= 1)
                    if pending is not None:
                        pkt, pea, peb, pvq = pending
                        nc.tensor.matmul(pva[:, pvq], vo[pkt][:, h0, :],
                                         pea[:, pvq],
                                         start=(pkt == 0), stop=False)
                        nc.tensor.matmul(pvb[:, pvq], vo[pkt][:, h1, :],
                                         peb[:, pvq],
                                         start=(pkt == 0), stop=False)
                    pending = (kt, ea, eb, vq)
                pkt, pea, peb, pvq = pending
                nc.tensor.matmul(pva[:, pvq], vo[pkt][:, h0, :],
                                 pea[:, pvq],
                                 start=(pkt == 0), stop=True)
                nc.tensor.matmul(pvb[:, pvq], vo[pkt][:, h1, :],
                                 peb[:, pvq],
                                 start=(pkt == 0), stop=True)
                def ready_work(hp=hp):
                    # chunk-0 stragglers, the previous chunk's out-proj, and
                    # chunk qc+1's projections -- all runnable immediately.
                    if qc == 0 and hp + 1 < 4 and INIT_STREAM:
                        cur_QT[hp + 1] = proj_q(0, hp + 1)
                        proj_k(0, hp + 1)
                    if prev is not None:
                        counts = {0: [2, 2, 2, 2], 1: [4, 4, 0, 0],
                                  2: [3, 3, 2, 0]}[OPDIST]
                        lo = sum(counts[:hp])
                        mts = range(lo, lo + counts[hp])
                        emit_outproj(prev[0], prev[1], mts)
                    if qc + 1 < NSC:
                        next_QT[hp] = proj_q(qc + 1, hp, alt=IROPE == 2)
                        proj_k(qc + 1, hp)
                        proj_v(qc + 1, hp, act=qc < VACT)
                    elif hp == 3:
                        # last block: prefill the final out-proj's first two
                        # column tiles over hp0-2 while hp3 still computes,
                        # so only the +hp3 matmul remains after its oT lands
                        for mt in (0, 1):
                            yps = pbps2.tile([128, SC], F32, tag=f"yps{tag}",
                                             name=f"yps{tag}", bufs=YBUFS)
                            ml = slice(mt * 128, (mt + 1) * 128)
                            for hp_ in range(3):
                                nc.tensor.matmul(yps, wo_sb[:, hp_, ml],
                                                 oTs[hp_], start=(hp_ == 0),
                                                 stop=False)
                            final_pre.append((mt, yps))

                if BLKORD == 1:
                    ready_work()
                # normalize: oT[j, q] = pv[j, q] / denom[q].
                # Stage PSUM -> SBUF (partition-aligned) so pva/pvb free
                # early; reciprocal reads the PSUM denom row directly
                # (in@p64 -> out@p0 is valid for single-input DVE ops);
                # broadcast to 64 partitions at base 0; head B's rows are
                # DMA-relocated to 64:128 (engine ops cannot cross-base).
                last_blk = qc == NSC - 1 and hp == 3
                d2 = pbt1.tile([1, 2, SC], F32, tag=f"d2{tag}", name=f"d2{tag}")
                # reciprocals first: they gate the broadcast+mul chain, the
                # copies don't
                nc.vector.reciprocal(d2[:, 0, :], pva[64:65, :])
                nc.vector.reciprocal(d2[:, 1, :], pvb[64:65, :])
                if last_blk:
                    # nothing needs the PSUM banks after this block: skip
                    # the staging copies, multiply straight from PSUM
                    o2a, o2b = pva[0:64, :], pvb[0:64, :]
                else:
                    o2 = pbt1.tile([65, 2, SC], F32, tag=f"o2{tag}", name=f"o2{tag}")
                    if qc < O2ACT:
                        # ScalarE has slack in the early chunks; late chunks
                        # are exp-bound so the copies go to DVE there.
                        nc.scalar.copy(o2[:, 0, :], pva)
                        nc.scalar.copy(o2[:, 1, :], pvb)
                    else:
                        nc.vector.tensor_copy(o2[:, 0, :], pva)
                        nc.vector.tensor_copy(o2[:, 1, :], pvb)
                    o2a, o2b = o2[0:64, 0, :], o2[0:64, 1, :]
                bc = pbt1.tile([64, 2, SC], F32, tag=f"bc{tag}", name=f"bc{tag}")
                nc.gpsimd.partition_broadcast(bc[:, 0, :], d2[:, 0, :])
                nc.gpsimd.partition_broadcast(bc[:, 1, :], d2[:, 1, :])
                oT = pbo.tile([128, SC], mmdt, tag=f"oT{hp}{tag}", name=f"oT{hp}{tag}")
                tmpB = pbt.tile([64, SC], mmdt, tag=f"tmpB{tag}", name=f"tmpB{tag}")
                nc.vector.tensor_mul(oT[0:64, :], o2a, bc[:, 0, :])
                nc.vector.tensor_mul(tmpB, o2b, bc[:, 1, :])
                if last_blk:
                    # head B stays at base 0; the final out-proj contracts
                    # it against wo3b instead of waiting for a relocate
                    last_tmpB[0] = tmpB
                else:
                    nc.sync.dma_start(out=oT[64:128, :], in_=tmpB)
                oTs.append(oT)
                if BLKORD == 0:
                    ready_work()
            prev = (qsl, oTs)
            if qc + 1 < NSC:
                cur_QT = next_QT
        # final out-proj: finish the prefilled tiles (+hp3 only), then the
        # rest; alternate ys staging between DVE and ScalarE so the last
        # copies drain in parallel
        qsl_, oTs_ = prev
        done_pre = {mt for mt, _ in final_pre}
        seq = [(mt, yps) for mt, yps in final_pre]
        seq += [(mt, None) for mt in range(8) if mt not in done_pre]
        for i, (mt, yps) in enumerate(seq):
            ml = slice(mt * 128, (mt + 1) * 128)
            if yps is None:
                yps = pbps2.tile([128, SC], F32, tag=f"yps{tag}",
                                 name=f"yps{tag}", bufs=YBUFS)
                for hp_ in range(3):
                    nc.tensor.matmul(yps, wo_sb[:, hp_, ml], oTs_[hp_],
                                     start=(hp_ == 0), stop=False)
            # hp3 contribution in two base-0 halves (no relocate needed)
            nc.tensor.matmul(yps, wo_sb[0:64, 3, ml], oTs_[3][0:64, :],
                             start=False, stop=False)
            nc.tensor.matmul(yps, wo3b[:, ml], last_tmpB[0],
                             start=False, stop=True)
            ys = pbt.tile([128, SC], BF16 if YBF16 else F32,
                          tag=f"ys{tag}", name=f"ys{tag}")
            if i % 2 == 0:
                nc.vector.tensor_copy(ys, yps)
            else:
                nc.scalar.copy(ys, yps)
            nc.sync.dma_start(out=yT[ml, qsl_], in_=ys)


_BUILT = {}


def build_nc(mmdt_name=MM_DTYPE, repeat=1):
    key = (mmdt_name, repeat)
    if key in _BUILT:
        return _BUILT[key]
    mmdt = {"f32": F32, "f32r": F32R, "bf16": BF16}[mmdt_name]
    nc = bacc.Bacc("TRN2", target_bir_lowering=False, debug=False,
                   num_devices=N_CORES)
    dram = {
        "xT": nc.dram_tensor("xT", [D, S], mmdt, kind="ExternalInput").ap(),
        "wq": nc.dram_tensor("wq", [D, JC], mmdt, kind="ExternalInput").ap(),
        "wk": nc.dram_tensor("wk", [D, JC], mmdt, kind="ExternalInput").ap(),
        "wv": nc.dram_tensor("wv", [D, JC], mmdt, kind="ExternalInput").ap(),
        "wo": nc.dram_tensor("wo", [JC, D], mmdt, kind="ExternalInput").ap(),
        "cosE": nc.dram_tensor("cosE", [128, S], F32,
                               kind="ExternalInput").ap(),
        "sinE": nc.dram_tensor("sinE", [128, S], F32,
                               kind="ExternalInput").ap(),
        "maskneg": nc.dram_tensor("maskneg", [128, 256], F32,
                                  kind="ExternalInput").ap(),
        "yT": nc.dram_tensor("yT", [D, S], BF16 if YBF16 else F32,
                             kind="ExternalOutput").ap(),
    }
    with tile.TileContext(nc) as tc:
        for r in range(repeat):
            _emit(nc, tc, mmdt, dram, tag=f"r{r}" if repeat > 1 else "")
    nc.compile()
    _BUILT[key] = nc
    return nc


def _round_f32r(a):
    """Round-to-nearest onto the f32r grid (fp32 with low 12 mantissa bits 0)."""
    b = np.ascontiguousarray(a, np.float32).view(np.uint32).astype(np.uint64)
    b = (b + 0x800 + ((b >> 12) & 1)) & 0xFFFFF000
    return b.astype(np.uint32).view(np.float32)


def _to_bf16(a):
    import ml_dtypes
    return np.ascontiguousarray(a, np.float32).astype(ml_dtypes.bfloat16)


def _host_prep(x, pos_ids, Wq, Wk, Wv, Wo, cos, sin, mmdt_name=None):
    """Build the 8 per-core input maps."""
    if mmdt_name is None:
        mmdt_name = MM_DTYPE
    if mmdt_name == "f32r":
        rnd = _round_f32r
    elif mmdt_name == "bf16":
        rnd = _to_bf16
    else:
        rnd = lambda a: a
    x = np.asarray(x, dtype=np.float32)
    pos_ids = np.asarray(pos_ids)
    cos = np.asarray(cos, dtype=np.float32)
    sin = np.asarray(sin, dtype=np.float32)
    freq_idx = np.tile(np.repeat(np.arange(DK // 2), 2), 2)  # [128]
    sign = np.where((np.arange(128) % 2) == 0, -1.0, 1.0).astype(np.float32)

    # universal triangular boundary mask: 1 if q >= p else 0 (multiplied
    # into exp(scores) post-activation); two side-by-side copies (one per
    # head in the merged [128, 2, 128] tile)
    p = np.arange(128)[:, None]
    q = np.arange(128)[None, :]
    mask1 = np.where(q >= p, 1.0, 0.0).astype(np.float32)
    mask = np.concatenate([mask1, mask1], axis=1)  # [128, 256]

    in_maps = []
    for c in range(N_CORES):
        b, g = c // 2, c % 2
        hs = slice(64 * HPC * g, 64 * HPC * g + JC)
        pos = pos_ids[b].astype(np.int64)
        cosT = cos[pos].T  # [32, S]
        sinT = sin[pos].T
        cosE = np.ascontiguousarray(cosT[freq_idx])           # [128, S]
        sinE = np.ascontiguousarray(sinT[freq_idx] * sign[:, None])
        in_maps.append({
            "xT": rnd(np.ascontiguousarray(x[b].T)),
            "wq": rnd(np.ascontiguousarray(Wq[hs, :].T)),
            "wk": rnd(np.ascontiguousarray(Wk[hs, :].T)),
            "wv": rnd(np.ascontiguousarray(Wv[hs, :].T)),
            "wo": rnd(np.ascontiguousarray(Wo[:, hs].T)),
            "cosE": cosE,
            "sinE": sinE,
            "maskneg": mask,
        })
    return in_maps


def kernel(x, pos_ids, Wq, Wk, Wv, Wo, cos, sin):
    nc = build_nc()
    in_maps = _host_prep(x, pos_ids, Wq, Wk, Wv, Wo, cos, sin)
    res = run_bass_kernel_spmd(nc, in_maps, list(range(N_CORES)))
    out = np.empty((B, S, D), dtype=np.float32)
    for b in range(B):
        yT = (np.asarray(res.results[2 * b]["yT"], dtype=np.float32)
              + np.asarray(res.results[2 * b + 1]["yT"], dtype=np.float32))
        out[b] = yT.T
    return out


if __name__ == "__main__":
    t0 = time.time()
    nc = build_nc()
    print(f"build+compile: {time.time()-t0:.1f}s", flush=True)
